# revision 11
# baseline (speedup 1.0000x reference)
"""DeepFM forward on 8 Trainium2 NeuronCores (Bass/Tile).

Strategy (batch-parallel, v3):
  - Host packs emb1/emb2 into one combined table of 17-float rows and
    flattens indices to (field * (V+1) + idx).
  - Each core owns 2048 samples. All 624 indirect-DMA gathers (16 tiles
    x 39 fields) are emitted first into 16 persistent SBUF tiles so the
    Pool/SWDGE engine streams them back-to-back with no buffer stalls.
  - Per tile (behind the gather stream): deep = [e2*xv | 1 | 0 0 0],
    bf16 copy, e1 + 0.5*s^2 accumulation on VectorE, and the extended
    Gram matrix G = [deep|1]^T [deep|1] accumulated on TensorE.
    G m-chunks 0-3 accumulate during the stream (8 PSUM banks); m-chunk
    4 runs after the stream. PSUM->SBUF flushes go to ScalarE as bf16.
  - BatchNorm statistics are LINEAR in G, so each core computes its
    local  mw1n = colsums(deep) @ W1  and  q1n = diag(W1^T G W1)  and
    only those (2x400 floats) are AllReduced -- no 628x628 G exchange.
        var1 = q1/N - (mw1/N)^2,  c1 = g1/sqrt(var1+eps)
        A2 = (W1 diag c1) W2
        var2 = q2/N - (ma2/N)^2,  c2 = g2/sqrt(var2+eps)   (2nd AllReduce)
        w = A2 c2,  C = -c2.(m A2) + sum(be2) + bias
    (b1, be1, b2 cancel exactly through the BN mean subtraction.)
  - Final: out = e1sum + 0.5*s^2 + deep.(w - 0.5*deep) + C' per sample,
    with C' = C + 0.5 compensating the ones-column; the -0.5*deep^2
    term supplies the FM -0.5*sum(e2^2) part.
"""
import os
import sys

for _p in ("/opt/trn_rl_repo", "/root/.axon_site/_ro/trn_rl_repo"):
    if os.path.isdir(_p) and _p not in sys.path:
        sys.path.insert(0, _p)

import numpy as np
import concourse.bacc as bacc
import concourse.tile as tile
import concourse.bass as bass
import concourse.bass_utils as bass_utils
from concourse import mybir
from concourse.masks import make_identity

N, F, V, E = 16384, 39, 100000, 16
ROWW = 17                      # 16 emb2 floats + 1 emb1 float per row
D = F * E                      # 624
DE = 628                       # 624 + ones column + 3 zero pad
H = 400
EPS = 1e-5
NCORES = 8
P = 128
NT = 16                        # tiles per core (2048 samples)

# k-chunks over the 624 deep dims (for 128-partition matmul tiles)
KCH = [(0, 128), (128, 256), (256, 384), (384, 512), (512, 624)]
# m-chunks over the 628 extended dims (G rows)
MCH = [(0, 128), (128, 256), (256, 384), (384, 512), (512, 628)]
# free-dim halves of G
NH = [(0, 320), (320, 628)]
# k-chunks over the 400 hidden dims
KC2 = [(0, 128), (128, 256), (256, 384), (384, 400)]

F32, I32 = mybir.dt.float32, mybir.dt.int32
BF16 = mybir.dt.bfloat16
MUL, ADD, SUB = (mybir.AluOpType.mult, mybir.AluOpType.add,
                 mybir.AluOpType.subtract)
AX = mybir.AxisListType.X


def _kernel_body(tc, outs, ins, spc):
    nc = tc.nc
    nt = spc // P
    assert nt == NT
    ctab, fidx, xv = ins["ctab"], ins["fidx"], ins["xv"]
    w1_in, w1t_in, w2_in = ins["W1"], ins["W1T"], ins["W2"]
    vecs, consts = ins["vecs"], ins["consts"]
    y = outs["y"]

    from contextlib import ExitStack
    ctx = ExitStack()
    sb = ctx.enter_context(tc.tile_pool(name="sb", bufs=1))
    sb2 = ctx.enter_context(tc.tile_pool(name="sb2", bufs=2))
    dr = ctx.enter_context(tc.tile_pool(name="dr", bufs=1, space="DRAM"))

    # ---- preload constants / weights / all indices ----
    ident = sb.tile([P, P], F32, tag="ident")
    make_identity(nc, ident[:])
    ones_row = sb.tile([1, P], F32, tag="ones_row")
    nc.vector.memset(ones_row[:], 1.0)
    ones_col = sb.tile([P, 1], BF16, tag="ones_col")
    nc.vector.memset(ones_col[:], 1.0)

    g1t = sb.tile([1, H], F32, tag="g1t")
    nc.sync.dma_start(out=g1t[:], in_=vecs[1:2, :])
    g2t = sb.tile([1, H], F32, tag="g2t")
    nc.sync.dma_start(out=g2t[:], in_=vecs[4:5, :])
    be2t = sb.tile([1, H], F32, tag="be2t")
    nc.sync.dma_start(out=be2t[:], in_=vecs[5:6, :])
    g1r, g2r, be2r = g1t[:], g2t[:], be2t[:]
    cst = sb.tile([1, 4], F32, tag="consts")
    nc.sync.dma_start(out=cst[:], in_=consts[:])
    BIAS, INVN, EPSC = cst[0:1, 0:1], cst[0:1, 1:2], cst[0:1, 2:3]

    # all indices / xv for the core, one DMA each:
    # fidx DRAM is [(t p), f] -> SBUF [p, (t f)]
    fidx_all = sb.tile([P, NT * F], I32, tag="fidx_all")
    nc.sync.dma_start(out=fidx_all[:].rearrange("p (t f) -> p t f", f=F),
                      in_=fidx[:].rearrange("(t p) f -> p t f", p=P))
    xv_all = sb.tile([P, NT * F], F32, tag="xv_all")
    nc.sync.dma_start(out=xv_all[:].rearrange("p (t f) -> p t f", f=F),
                      in_=xv[:].rearrange("(t p) f -> p t f", p=P))

    w1f, w1b = [], []
    for i, (lo, hi) in enumerate(KCH):
        t = sb.tile([hi - lo, H], F32, tag=f"w1f{i}")
        nc.sync.dma_start(out=t[:], in_=w1_in[lo:hi, :])
        r = sb.tile([hi - lo, H], BF16, tag=f"w1b{i}")
        nc.vector.tensor_copy(out=r[:], in_=t[:])
        w1f.append(t)
        w1b.append(r)
    w1tf = []
    for i, (lo, hi) in enumerate(KC2):
        t = sb.tile([hi - lo, D], F32, tag=f"w1tf{i}")
        nc.sync.dma_start(out=t[:], in_=w1t_in[lo:hi, :])
        w1tf.append(t)
    w2f = []
    for i, (lo, hi) in enumerate(KC2):
        t = sb.tile([hi - lo, H], F32, tag=f"w2f{i}")
        nc.sync.dma_start(out=t[:], in_=w2_in[lo:hi, :])
        w2f.append(t)

    # ---- the gather stream: all 624 indirect DMAs, back-to-back ----
    raws = [sb.tile([P, F * ROWW], F32, tag=f"raw{t}", name=f"raw{t}")
            for t in range(nt)]
    for t in range(nt):
        for f in range(F):
            nc.gpsimd.indirect_dma_start(
                out=raws[t][:, f * ROWW:(f + 1) * ROWW],
                out_offset=None,
                in_=ctab[:],
                in_offset=bass.IndirectOffsetOnAxis(
                    ap=fidx_all[:, t * F + f:t * F + f + 1], axis=0),
            )

    # ---- per-tile work behind the stream ----
    deep = [sb.tile([P, DE], F32, tag=f"deep{t}", name=f"deep{t}")
            for t in range(nt)]
    deepb = [sb.tile([P, DE], BF16, tag=f"deepb{t}", name=f"deepb{t}")
             for t in range(nt)]
    acc = sb.tile([P, nt], F32, tag="acc")

    gps = {}

    def g_matmuls(t, ms):
        mlo, mhi = MCH[ms]
        for h_ in range(2):
            nlo, nhi = NH[h_]
            if (ms, h_) not in gps:
                gps[(ms, h_)] = psg.tile([P, 320], F32, tag="gps",
                                         name=f"gps{ms}_{h_}")
            nc.tensor.matmul(
                out=gps[(ms, h_)][0:mhi - mlo, : nhi - nlo],
                lhsT=deepb[t][:, mlo:mhi],
                rhs=deepb[t][:, nlo:nhi],
                start=(t == 0), stop=(t == nt - 1),
            )

    psg_cm = tc.tile_pool(name="psg", bufs=8, space="PSUM")
    psg = psg_cm.__enter__()

    for t in range(nt):
        xv_t = xv_all[:, t * F:(t + 1) * F]
        raw3 = raws[t][:].rearrange("p (f r) -> p f r", r=ROWW)

        # deep tile: scaled e2 + ones column + zero pad
        nc.vector.tensor_scalar(
            out=deep[t][:, D:D + 1], in0=xv_t[:, 0:1], scalar1=0.0, scalar2=1.0,
            op0=MUL, op1=ADD)
        nc.vector.tensor_scalar(
            out=deep[t][:, D + 1:DE], in0=xv_t[:, 0:3], scalar1=0.0,
            scalar2=None, op0=MUL)
        nc.vector.tensor_tensor(
            out=deep[t][:, 0:D].rearrange("p (f e) -> p f e", e=E),
            in0=raw3[:, :, 0:E],
            in1=xv_t.unsqueeze(-1).to_broadcast([P, F, E]),
            op=MUL)
        nc.vector.tensor_copy(out=deepb[t][:], in_=deep[t][:])
        dview = deep[t][:, 0:D]

        # e1 contribution
        e1v = sb2.tile([P, F], F32, tag="e1v")
        nc.vector.tensor_tensor(out=e1v[:], in0=raw3[:, :, E], in1=xv_t, op=MUL)
        nc.vector.reduce_sum(out=acc[:, t:t + 1], in_=e1v[:], axis=AX)

        # + 0.5 * sum_e s_e^2   (the -0.5*sum deep^2 part is folded into
        # the final dot product)
        s16 = sb2.tile([P, E], F32, tag="s16")
        nc.vector.reduce_sum(
            out=s16[:], in_=dview.rearrange("p (f e) -> p e f", e=E), axis=AX)
        nc.vector.tensor_tensor(out=s16[:], in0=s16[:], in1=s16[:], op=MUL)
        fm1 = sb2.tile([P, 1], F32, tag="fm1")
        nc.vector.reduce_sum(out=fm1[:], in_=s16[:], axis=AX)
        nc.vector.tensor_scalar(out=fm1[:], in0=fm1[:], scalar1=0.5,
                                scalar2=None, op0=MUL)
        nc.vector.tensor_tensor(out=acc[:, t:t + 1], in0=acc[:, t:t + 1],
                                in1=fm1[:], op=ADD)

        # G m-chunks 0..3 accumulate during the stream (8 PSUM buffers)
        for ms in (0, 1, 2, 3):
            g_matmuls(t, ms)

    # ---- post-stream: remaining G waves, flush to SBUF bf16 ----
    grr = {}        # local G rows, bf16, chunked like MCH

    def flush_chunk(ms):
        mlo, mhi = MCH[ms]
        gsb = sb.tile([P, DE], BF16, tag=f"grr{ms}", name=f"grr{ms}")
        for h_ in range(2):
            nlo, nhi = NH[h_]
            nc.scalar.copy(out=gsb[0:mhi - mlo, nlo:nhi],
                           in_=gps[(ms, h_)][0:mhi - mlo, : nhi - nlo])
            del gps[(ms, h_)]
        grr[ms] = gsb

    for ms in (0, 1, 2, 3):
        flush_chunk(ms)
    for t in range(nt):
        g_matmuls(t, 4)
    flush_chunk(4)

    psg_cm.__exit__(None, None, None)
    pstat_cm = tc.tile_pool(name="pstat", bufs=2, space="PSUM")
    pstat = pstat_cm.__enter__()
    psmall_cm = tc.tile_pool(name="psmall", bufs=2, space="PSUM")
    psmall = psmall_cm.__enter__()

    def g_rows(kt):  # G k-rows (excluding the ones row 624)
        lo, hi = KCH[kt]
        return grr[kt][0:hi - lo, :] if kt < 4 else grr[4][0:112, :]

    # ---- layer-1 local stats: mw1n (colsums @ W1) and q1n = diag(W1^T G W1)
    mw1n = psmall.tile([1, H], F32, tag="psm")
    for kt in range(5):
        nc.tensor.matmul(
            out=mw1n[:], lhsT=g_rows(kt)[:, D:D + 1], rhs=w1b[kt][:],
            start=(kt == 0), stop=(kt == 4))
    q1n = psmall.tile([1, H], F32, tag="psm")
    prods = []
    for kt in range(5):
        gw = pstat.tile([KCH[kt][1] - KCH[kt][0], H], F32, tag="pstat")
        for kt2 in range(5):
            nc.tensor.matmul(
                out=gw[:], lhsT=g_rows(kt2)[:, KCH[kt][0]:KCH[kt][1]],
                rhs=w1b[kt2][:], start=(kt2 == 0), stop=(kt2 == 4))
        pr = sb2.tile([KCH[kt][1] - KCH[kt][0], H], BF16, tag="prod")
        nc.vector.tensor_tensor(out=pr[:], in0=w1f[kt][:], in1=gw[:], op=MUL)
        prods.append(pr)
    for kt in range(5):
        nc.tensor.matmul(
            out=q1n[:], lhsT=ones_col[0:KCH[kt][1] - KCH[kt][0], :],
            rhs=prods[kt][:], start=(kt == 0), stop=(kt == 4))

    # ---- AllReduce #1: [mw1n | q1n]  (800 floats) ----
    st1 = dr.tile([1, 2 * H], F32)
    st1r = dr.tile([1, 2 * H], F32)
    arow = sb.tile([1, 2 * H], F32, tag="arow")
    nc.scalar.copy(out=arow[0:1, 0:H], in_=mw1n[:])
    nc.scalar.copy(out=arow[0:1, H:2 * H], in_=q1n[:])
    nc.sync.dma_start(out=st1[:], in_=arow[:])
    nc.gpsimd.collective_compute(
        "AllReduce", ADD, replica_groups=[list(range(NCORES))],
        ins=[st1.opt()], outs=[st1r.opt()])
    arow1 = sb.tile([1, 2 * H], F32, tag="arow1")
    nc.sync.dma_start(out=arow1[:], in_=st1r[:])

    # ---- c1 = g1 * rsqrt(var1 + eps) ----
    mw1row = sb.tile([1, H], F32, tag="mw1row")
    var1row = sb.tile([1, H], F32, tag="var1row")
    c1row = sb.tile([1, H], F32, tag="c1row")
    c2row = sb.tile([1, H], F32, tag="c2row")
    nc.vector.tensor_scalar(out=mw1row[:], in0=arow1[0:1, 0:H], scalar1=INVN,
                            scalar2=None, op0=MUL)
    nc.vector.tensor_scalar(out=var1row[:], in0=arow1[0:1, H:2 * H],
                            scalar1=INVN, scalar2=None, op0=MUL)
    tmp1 = sb.tile([1, H], F32, tag="tmp1")
    tmp2 = sb.tile([1, H], F32, tag="tmp2")
    nc.vector.tensor_tensor(out=tmp1[:], in0=mw1row[:], in1=mw1row[:], op=MUL)
    nc.vector.tensor_tensor(out=var1row[:], in0=var1row[:], in1=tmp1[:], op=SUB)
    nc.scalar.activation(out=tmp1[:], in_=var1row[:],
                         func=mybir.ActivationFunctionType.Sqrt, bias=EPSC)
    nc.vector.reciprocal(out=tmp2[:], in_=tmp1[:])
    nc.vector.tensor_tensor(out=tmp1[:], in0=tmp1[:], in1=tmp2[:], op=MUL)
    nc.vector.tensor_scalar(out=tmp1[:], in0=tmp1[:], scalar1=-1.0, scalar2=2.0,
                            op0=MUL, op1=ADD)
    nc.vector.tensor_tensor(out=tmp2[:], in0=tmp2[:], in1=tmp1[:], op=MUL)
    nc.vector.tensor_tensor(out=c1row[:], in0=g1r, in1=tmp2[:], op=MUL)

    # c1 column layout via DRAM bounce
    c1d = dr.tile([H], F32)
    nc.sync.dma_start(out=c1d[:], in_=c1row[:])
    c1c = sb.tile([P, 4], F32, tag="c1c")
    for i, (lo, hi) in enumerate(KC2):
        nc.sync.dma_start(out=c1c[0:hi - lo, i:i + 1],
                          in_=c1d[lo:hi].unsqueeze(-1))

    # A2 = (W1 diag c1) W2
    w2r_t, a2b, a2f = [], [], []
    for i, (lo, hi) in enumerate(KC2):
        sc = sb.tile([hi - lo, H], BF16, tag=f"w2r{i}")
        nc.vector.tensor_scalar(out=sc[:], in0=w2f[i][:],
                                scalar1=c1c[0:hi - lo, i:i + 1], scalar2=None,
                                op0=MUL)
        w2r_t.append(sc)
        w1ts = sb.tile([hi - lo, D], BF16, tag=f"w1ts{i}")
        nc.vector.tensor_scalar(out=w1ts[:], in0=w1tf[i][:],
                                scalar1=c1c[0:hi - lo, i:i + 1], scalar2=None,
                                op0=MUL)
        w1tf[i] = w1ts
    for ms in range(5):
        mlo, mhi = KCH[ms]
        ap2 = pstat.tile([mhi - mlo, H], F32, tag="pstat")
        for kt2 in range(4):
            nc.tensor.matmul(
                out=ap2[:], lhsT=w1tf[kt2][:, mlo:mhi], rhs=w2r_t[kt2][:],
                start=(kt2 == 0), stop=(kt2 == 3))
        af = sb.tile([mhi - mlo, H], F32, tag=f"a2f{ms}")
        nc.scalar.copy(out=af[:], in_=ap2[:])
        ar = sb.tile([mhi - mlo, H], BF16, tag=f"a2b{ms}")
        nc.vector.tensor_copy(out=ar[:], in_=ap2[:])
        a2f.append(af)
        a2b.append(ar)

    # ---- layer-2 local stats ----
    ma2n = psmall.tile([1, H], F32, tag="psm")
    for kt in range(5):
        nc.tensor.matmul(
            out=ma2n[:], lhsT=g_rows(kt)[:, D:D + 1],
            rhs=a2b[kt][:], start=(kt == 0), stop=(kt == 4))
    q2n = psmall.tile([1, H], F32, tag="psm")
    prods2 = []
    for kt in range(5):
        gw = pstat.tile([KCH[kt][1] - KCH[kt][0], H], F32, tag="pstat")
        for kt2 in range(5):
            nc.tensor.matmul(
                out=gw[:], lhsT=g_rows(kt2)[:, KCH[kt][0]:KCH[kt][1]],
                rhs=a2b[kt2][:], start=(kt2 == 0), stop=(kt2 == 4))
        pr = sb2.tile([KCH[kt][1] - KCH[kt][0], H], BF16, tag="prod")
        nc.vector.tensor_tensor(out=pr[:], in0=a2f[kt][:], in1=gw[:], op=MUL)
        prods2.append(pr)
    for kt in range(5):
        nc.tensor.matmul(
            out=q2n[:], lhsT=ones_col[0:KCH[kt][1] - KCH[kt][0], :],
            rhs=prods2[kt][:], start=(kt == 0), stop=(kt == 4))

    # ---- AllReduce #2: [ma2n | q2n] ----
    st2 = dr.tile([1, 2 * H], F32)
    st2r = dr.tile([1, 2 * H], F32)
    brow = sb.tile([1, 2 * H], F32, tag="brow")
    nc.scalar.copy(out=brow[0:1, 0:H], in_=ma2n[:])
    nc.scalar.copy(out=brow[0:1, H:2 * H], in_=q2n[:])
    nc.sync.dma_start(out=st2[:], in_=brow[:])
    nc.gpsimd.collective_compute(
        "AllReduce", ADD, replica_groups=[list(range(NCORES))],
        ins=[st2.opt()], outs=[st2r.opt()])
    brow1 = sb.tile([1, 2 * H], F32, tag="brow1")
    nc.sync.dma_start(out=brow1[:], in_=st2r[:])

    ma2 = sb.tile([1, H], F32, tag="ma2")
    nc.vector.tensor_scalar(out=ma2[:], in0=brow1[0:1, 0:H], scalar1=INVN,
                            scalar2=None, op0=MUL)
    var2 = sb.tile([1, H], F32, tag="var2")
    nc.vector.tensor_scalar(out=var2[:], in0=brow1[0:1, H:2 * H], scalar1=INVN,
                            scalar2=None, op0=MUL)
    nc.vector.tensor_tensor(out=tmp1[:], in0=ma2[:], in1=ma2[:], op=MUL)
    nc.vector.tensor_tensor(out=var2[:], in0=var2[:], in1=tmp1[:], op=SUB)
    nc.scalar.activation(out=tmp1[:], in_=var2[:],
                         func=mybir.ActivationFunctionType.Sqrt, bias=EPSC)
    nc.vector.reciprocal(out=tmp2[:], in_=tmp1[:])
    nc.vector.tensor_tensor(out=tmp1[:], in0=tmp1[:], in1=tmp2[:], op=MUL)
    nc.vector.tensor_scalar(out=tmp1[:], in0=tmp1[:], scalar1=-1.0, scalar2=2.0,
                            op0=MUL, op1=ADD)
    nc.vector.tensor_tensor(out=tmp2[:], in0=tmp2[:], in1=tmp1[:], op=MUL)
    nc.vector.tensor_tensor(out=c2row[:], in0=g2r, in1=tmp2[:], op=MUL)

    # ---- w = A2 c2 (column), w_row = [w | C'], wb broadcast ----
    c2bp = pstat.tile([P, H], F32, tag="pstat")
    nc.tensor.matmul(out=c2bp[:], lhsT=ones_row[:], rhs=c2row[:],
                     start=True, stop=True)
    c2b = sb.tile([P, H], F32, tag="c2b")
    nc.scalar.copy(out=c2b[:], in_=c2bp[:])

    w_row = sb.tile([1, DE], F32, tag="w_row")
    nc.vector.memset(w_row[:], 0.0)
    for ms in range(5):
        mlo, mhi = KCH[ms]
        wc = sb2.tile([mhi - mlo, 1], F32, tag="wcol")
        prw = sb2.tile([mhi - mlo, H], F32, tag="prw")
        nc.vector.tensor_tensor(out=prw[:], in0=a2f[ms][:],
                                in1=c2b[0:mhi - mlo, :], op=MUL)
        nc.vector.reduce_sum(out=wc[:], in_=prw[:], axis=AX)
        wrp = psmall.tile([1, P], F32, tag="psm")
        nc.tensor.transpose(out=wrp[0:1, 0:mhi - mlo], in_=wc[:],
                            identity=ident[0:mhi - mlo, 0:mhi - mlo])
        nc.vector.tensor_copy(out=w_row[0:1, mlo:mhi], in_=wrp[0:1, 0:mhi - mlo])

    # C' = -c2.mA2 + sum(be2) + bias + 0.5  -> w_row[0, 624]
    nc.vector.tensor_tensor(out=tmp1[:], in0=c2row[:], in1=ma2[:], op=MUL)
    csum = sb.tile([1, 2], F32, tag="csum")
    nc.vector.reduce_sum(out=csum[0:1, 0:1], in_=tmp1[:], axis=AX)
    nc.vector.reduce_sum(out=csum[0:1, 1:2], in_=be2r, axis=AX)
    nc.vector.tensor_tensor(out=csum[0:1, 1:2], in0=csum[0:1, 1:2],
                            in1=csum[0:1, 0:1], op=SUB)
    nc.vector.tensor_scalar(out=w_row[0:1, D:D + 1], in0=csum[0:1, 1:2],
                            scalar1=BIAS, scalar2=None, op0=ADD)

    wb = sb.tile([P, DE], F32, tag="wb")
    for h_ in range(2):
        nlo, nhi = NH[h_]
        bp = pstat.tile([P, 320], F32, tag="pstat")
        nc.tensor.matmul(out=bp[:, : nhi - nlo], lhsT=ones_row[:],
                         rhs=w_row[0:1, nlo:nhi], start=True, stop=True)
        nc.scalar.copy(out=wb[:, nlo:nhi], in_=bp[:, : nhi - nlo])

    # ---- final: out = acc + deep.(wb - 0.5*deep) ----
    yv = y[:].rearrange("(t p) -> p t", p=P)
    for t in range(nt):
        pr = sb2.tile([P, DE], F32, tag="fprod")
        nc.vector.tensor_scalar(out=pr[:], in0=deep[t][:], scalar1=-0.5,
                                scalar2=None, op0=MUL)
        nc.vector.tensor_tensor(out=pr[:], in0=pr[:], in1=wb[:], op=ADD)
        nc.vector.tensor_tensor(out=pr[:], in0=pr[:], in1=deep[t][:], op=MUL)
        dt_ = sb2.tile([P, 1], F32, tag="dtot")
        nc.vector.reduce_sum(out=dt_[:], in_=pr[:], axis=AX)
        nc.vector.tensor_tensor(out=acc[:, t:t + 1], in0=acc[:, t:t + 1],
                                in1=dt_[:], op=ADD)
    nc.sync.dma_start(out=yv, in_=acc[:])
    psmall_cm.__exit__(None, None, None)
    pstat_cm.__exit__(None, None, None)
    ctx.close()


_CACHED = {}


def _build(spc):
    key = spc
    if key in _CACHED:
        return _CACHED[key]
    nc = bacc.Bacc("TRN2", target_bir_lowering=False, debug=False,
                   enable_asserts=True, num_devices=NCORES)
    ins = {}
    outs = {}
    ispec = dict(
        ctab=((F * (V + 1), ROWW), F32),
        fidx=((spc, F), I32),
        xv=((spc, F), F32),
        W1=((D, H), F32),
        W1T=((H, D), F32),
        W2=((H, H), F32),
        vecs=((6, H), F32),
        consts=((1, 4), F32),
    )
    for name, (shape, dt) in ispec.items():
        ins[name] = nc.dram_tensor(name, list(shape), dt, kind="ExternalInput").ap()
    outs["y"] = nc.dram_tensor("y", [spc], F32, kind="ExternalOutput").ap()
    with tile.TileContext(nc) as tc:
        _kernel_body(tc, outs, ins, spc)
    nc.finalize()
    _CACHED[key] = nc
    return nc


def run_sharded(inputs, trace=False, trace_cores=None):
    """Shard full inputs, run on 8 cores, return (full_output, results)."""
    xi = np.asarray(inputs["xi"])
    xv = np.ascontiguousarray(np.asarray(inputs["xv"], dtype=np.float32))
    emb1 = np.asarray(inputs["emb1"], dtype=np.float32)
    emb2 = np.asarray(inputs["emb2"], dtype=np.float32)
    W1 = np.ascontiguousarray(np.asarray(inputs["W1"], dtype=np.float32))
    W2 = np.ascontiguousarray(np.asarray(inputs["W2"], dtype=np.float32))
    vecs = np.stack([
        np.asarray(inputs["b1"], dtype=np.float32),
        np.asarray(inputs["g1"], dtype=np.float32),
        np.asarray(inputs["be1"], dtype=np.float32),
        np.asarray(inputs["b2"], dtype=np.float32),
        np.asarray(inputs["g2"], dtype=np.float32),
        np.asarray(inputs["be2"], dtype=np.float32),
    ]).astype(np.float32)
    bias = float(np.asarray(inputs["bias"]).reshape(-1)[0])

    n = xi.shape[0]
    spc = n // NCORES
    ctab = np.empty((F * (V + 1), ROWW), np.float32)
    ctab[:, :E] = emb2.reshape(F * (V + 1), E)
    ctab[:, E] = emb1.reshape(F * (V + 1))
    fidx = (np.arange(F, dtype=np.int64)[None, :] * (V + 1)
            + xi[:, :, 0].astype(np.int64)).astype(np.int32)
    consts = np.array([[bias + 0.5, 1.0 / n, EPS, 0.0]], np.float32)
    W1T = np.ascontiguousarray(W1.T)

    nc = _build(spc)
    in_maps = []
    for c in range(NCORES):
        sl = slice(c * spc, (c + 1) * spc)
        in_maps.append(dict(
            ctab=ctab, fidx=np.ascontiguousarray(fidx[sl]),
            xv=np.ascontiguousarray(xv[sl]), W1=W1, W1T=W1T, W2=W2,
            vecs=vecs, consts=consts))
    res = bass_utils.run_bass_kernel_spmd(
        nc, in_maps, core_ids=list(range(NCORES)), trace=trace,
        trace_cores=trace_cores)
    out = np.concatenate([res.results[c]["y"] for c in range(NCORES)])
    return out, res


def kernel(**inputs):
    out, _ = run_sharded(inputs, trace=False)
    return out
